# revision 14
# baseline (speedup 1.0000x reference)
"""Trainium2 Bass kernel for windowed embedding lookup (nn_AttentionLayer).

Computation:
  out[b,s,e] = sum_k w[k,e] * data[snip_b, clip(inputs[b,s]+k-5, 0, 165), 0, e]

Strategy (data-parallel over batch, 2 batches per core on 8 cores):
  1. Per batch, gather the snippet's table slice T [166,768] (transposed
     [e,p] layout, staged by the host) via indirect DMA.
  2. Compute the 11-tap clip-padded convolution C[p,e] = sum_k w[k,e]*T[clip(p+k-5),e]
     on DVE/ACT (per-partition-scalar products + tensor adds).
  3. Transpose C back to [p,e] via TensorE.
  4. Gather rows out[s] = C[inputs[s]] as a one-hot matmul on TensorE.
  5. DMA the [1126,768] result straight to DRAM.

The host only does layout transforms (slice/transpose/reshape) and sharding;
all arithmetic runs on device.
"""

import sys

for _p in ("/opt/trn_rl_repo",):
    if _p not in sys.path:
        sys.path.insert(0, _p)

import numpy as np

N_CORES = 8
B = 16
BPC = B // N_CORES  # batches per core
S = 1126
E = 768
EC = 6  # number of 128-wide e chunks
P = 166  # table positions
PPAD = 176  # padded positions (5 on each side)
W = 11
NSNIP = 100
MTILES = (S + 127) // 128  # 9

_cache = {}


def _build(debug=False):
    import concourse.bass as bass
    import concourse.mybir as mybir
    import concourse.tile as tile
    from concourse import bacc
    from concourse.masks import make_identity

    f32 = mybir.dt.float32
    i32 = mybir.dt.int32
    AOT = mybir.AluOpType

    nc = bacc.Bacc()
    dbg = {}
    if debug:
        dbg["t2"] = nc.declare_dram_parameter(
            "dbg_t2", [128, EC * PPAD], f32, isOutput=True
        )
        dbg["rows"] = nc.declare_dram_parameter(
            "dbg_rows", [128, 1], i32, isOutput=True
        )
        dbg["inpb"] = nc.declare_dram_parameter(
            "dbg_inpb", [128, S], f32, isOutput=True
        )
        dbg["oh0"] = nc.declare_dram_parameter(
            "dbg_oh0", [128, S], f32, isOutput=True
        )
        dbg["c2"] = nc.declare_dram_parameter(
            "dbg_c2", [128, EC * P], f32, isOutput=True
        )
        dbg["ccat0"] = nc.declare_dram_parameter(
            "dbg_ccat0", [128, E], f32, isOutput=True
        )

    inp = nc.declare_dram_parameter("inp", [1, BPC * S], i32, isOutput=False)
    snip = nc.declare_dram_parameter("snip", [1, BPC], i32, isOutput=False)
    # row (snip*128 + i) holds [c*166 + p] -> data[snip, p, 0, c*128 + i]
    dataT2 = nc.declare_dram_parameter(
        "dataT2", [NSNIP * 128, EC * P], f32, isOutput=False
    )
    # [i, c*11 + k] -> w[k, c*128 + i]
    w2 = nc.declare_dram_parameter("w2", [128, EC * W], f32, isOutput=False)
    out = nc.declare_dram_parameter("out", [BPC * S, E], f32, isOutput=True)

    with tile.TileContext(nc) as tc:
        with (
            tc.tile_pool(name="const", bufs=1) as constp,
            tc.tile_pool(name="work", bufs=2) as workp,
            tc.tile_pool(name="mm", bufs=2) as mmp,
            tc.tile_pool(name="ob", bufs=3) as obp,
            tc.tile_pool(name="psum_t", bufs=2, space="PSUM") as psumt,
            tc.tile_pool(name="psum_mm", bufs=2, space="PSUM") as psummm,
        ):
            ident = constp.tile([128, 128], f32)
            make_identity(nc, ident[:])

            ones1 = constp.tile([1, 128], f32)
            nc.gpsimd.memset(ones1[:], 1.0)

            iota_i = constp.tile([128, 1], i32)
            nc.gpsimd.iota(iota_i[:], [[1, 1]], base=0, channel_multiplier=1)
            iota_f = constp.tile([128, 1], f32)
            nc.vector.tensor_copy(iota_f[:], iota_i[:])
            iota_f_hi = constp.tile([128, 1], f32)
            nc.vector.tensor_scalar_add(iota_f_hi[:], iota_f[:], 128.0)

            w2t = constp.tile([128, EC * W], f32)
            nc.sync.dma_start(out=w2t[:], in_=w2[:])
            inpt = constp.tile([1, BPC * S], i32)
            nc.sync.dma_start(out=inpt[:], in_=inp[:])
            snipt = constp.tile([1, BPC], i32)
            nc.sync.dma_start(out=snipt[:], in_=snip[:])

            for b in range(BPC):
                # ---- table row indices: snip*128 + iota (per-partition)
                snipf = workp.tile([1, 1], f32, tag="snipf")
                nc.vector.tensor_copy(snipf[:], snipt[0:1, b : b + 1])
                ps_sn = psumt.tile([128, 512], f32, tag="tp")
                nc.tensor.matmul(
                    out=ps_sn[:, :1],
                    lhsT=ones1[:, :],
                    rhs=snipf[:, :],
                    start=True,
                    stop=True,
                )
                rows_f = workp.tile([128, 1], f32, tag="rowsf")
                nc.vector.tensor_scalar(
                    rows_f[:], ps_sn[:, :1], 128.0, iota_f[:, :1],
                    AOT.mult, AOT.add,
                )
                rows2 = workp.tile([128, 1], i32, tag="rows2")
                nc.vector.tensor_copy(rows2[:], rows_f[:])
                if debug and b == 0:
                    nc.sync.dma_start(out=dbg["rows"][:], in_=rows2[:])

                # ---- gather the snippet table slice (contiguous dest), then
                # spread into the padded [e,p] layout
                t2raw = workp.tile([128, EC * P], f32, tag="t2raw")
                nc.gpsimd.indirect_dma_start(
                    out=t2raw[:, :],
                    out_offset=None,
                    in_=dataT2[:],
                    in_offset=bass.IndirectOffsetOnAxis(ap=rows2[:, :1], axis=0),
                )
                t2 = workp.tile([128, EC, PPAD], f32, tag="t2")
                nc.vector.tensor_copy(
                    t2[:, :, 5 : 5 + P],
                    t2raw[:, :].rearrange("p (c q) -> p c q", q=P),
                )
                for c in range(EC):
                    nc.vector.tensor_copy(
                        t2[:, c, 0:5], t2[:, c, 5:6].to_broadcast([128, 5])
                    )
                    nc.vector.tensor_copy(
                        t2[:, c, 5 + P : PPAD],
                        t2[:, c, 4 + P : 5 + P].to_broadcast([128, 5]),
                    )
                if debug and b == 0:
                    nc.sync.dma_start(out=dbg["t2"][:], in_=t2[:].rearrange("p c q -> p (c q)"))

                # ---- 11-tap conv: accA = sum_{k even}, accB = sum_{k odd}
                accA = workp.tile([128, EC, P], f32, tag="accA")
                accB = workp.tile([128, EC, P], f32, tag="accB")
                # k = 0 / k = 1 write straight into the accumulators
                for c in range(EC):
                    nc.vector.tensor_scalar(
                        accA[:, c, :], t2[:, c, 0:P],
                        w2t[:, c * W : c * W + 1], None, AOT.mult,
                    )
                    nc.scalar.activation(
                        accB[:, c, :], t2[:, c, 1 : 1 + P],
                        mybir.ActivationFunctionType.Copy,
                        scale=w2t[:, c * W + 1 : c * W + 2],
                    )
                for k in range(2, W):
                    prod = workp.tile([128, EC, P], f32, tag=f"prod{k % 4}")
                    for c in range(EC):
                        if k % 2 == 0:
                            nc.vector.tensor_scalar(
                                prod[:, c, :], t2[:, c, k : k + P],
                                w2t[:, c * W + k : c * W + k + 1], None, AOT.mult,
                            )
                        else:
                            nc.scalar.activation(
                                prod[:, c, :], t2[:, c, k : k + P],
                                mybir.ActivationFunctionType.Copy,
                                scale=w2t[:, c * W + k : c * W + k + 1],
                            )
                    if k % 2 == 0:
                        nc.vector.tensor_add(accA[:], accA[:], prod[:])
                    else:
                        nc.gpsimd.tensor_add(accB[:], accB[:], prod[:])
                c2 = workp.tile([128, EC, P], f32, tag="c2")
                nc.vector.tensor_add(c2[:], accA[:], accB[:])
                if debug and b == 0:
                    nc.sync.dma_start(out=dbg["c2"][:], in_=c2[:].rearrange("p c q -> p (c q)"))

                # ---- transpose C2 [e,p] -> C [p,e] (2 partition chunks)
                ccat0 = mmp.tile([128, E], f32, tag="c0")
                ccat1 = mmp.tile([128, E], f32, tag="c1")
                nc.vector.memzero(ccat1[:])
                for c in range(EC):
                    ps0 = psumt.tile([128, 512], f32, tag="tp")
                    nc.tensor.transpose(ps0[:, :128], c2[:, c, 0:128], ident[:])
                    nc.vector.tensor_copy(
                        ccat0[:, c * 128 : (c + 1) * 128], ps0[:, :128]
                    )
                    ps1 = psumt.tile([128, 512], f32, tag="tp")
                    nc.tensor.transpose(ps1[:38, :128], c2[:, c, 128:P], ident[:])
                    nc.scalar.copy(
                        ccat1[:38, c * 128 : (c + 1) * 128], ps1[:38, :128]
                    )
                if debug and b == 0:
                    nc.sync.dma_start(out=dbg["ccat0"][:], in_=ccat0[:])

                # ---- one-hot build (transposed): oh[p, s] = (inputs[s] == p)
                # replicate the inputs row across partitions via ones-matmul
                inpr_f = workp.tile([1, S], f32, tag="inprf")
                nc.vector.tensor_copy(
                    inpr_f[:], inpt[0:1, b * S : (b + 1) * S]
                )
                inpb_f = workp.tile([128, S], f32, tag="inpbf")
                for n0 in range(0, S, 512):
                    nw = min(512, S - n0)
                    ps_in = psumt.tile([128, 512], f32, tag="tp")
                    nc.tensor.matmul(
                        out=ps_in[:, :nw],
                        lhsT=ones1[:, :],
                        rhs=inpr_f[:, n0 : n0 + nw],
                        start=True,
                        stop=True,
                    )
                    nc.vector.tensor_copy(
                        inpb_f[:, n0 : n0 + nw], ps_in[:, :nw]
                    )
                oh0 = mmp.tile([128, S], f32, tag="oh0")
                oh1 = mmp.tile([128, S], f32, tag="oh1")
                nc.vector.tensor_scalar(
                    oh0[:], inpb_f[:], iota_f[:, :1], None, AOT.is_equal
                )
                nc.vector.tensor_scalar(
                    oh1[:], inpb_f[:], iota_f_hi[:, :1], None, AOT.is_equal
                )
                if debug and b == 0:
                    nc.sync.dma_start(out=dbg["inpb"][:], in_=inpb_f[:])
                    nc.sync.dma_start(out=dbg["oh0"][:], in_=oh0[:])

                # ---- gather matmul: out[s, e] = sum_p oh[p, s] * C[p, e]
                for m in range(MTILES):
                    mw = min(128, S - m * 128)
                    pso = psummm.tile([128, E], f32, tag="po")
                    for n0, nw in ((0, 512), (512, 256)):
                        nc.tensor.matmul(
                            out=pso[:mw, n0 : n0 + nw],
                            lhsT=oh0[:, m * 128 : m * 128 + mw],
                            rhs=ccat0[:, n0 : n0 + nw],
                            start=True,
                            stop=False,
                        )
                        nc.tensor.matmul(
                            out=pso[:mw, n0 : n0 + nw],
                            lhsT=oh1[:, m * 128 : m * 128 + mw],
                            rhs=ccat1[:, n0 : n0 + nw],
                            start=False,
                            stop=True,
                        )
                    ob = obp.tile([128, E], f32, tag="ob")
                    if m % 2 == 0:
                        nc.vector.tensor_copy(ob[:mw, :], pso[:mw, :])
                    else:
                        nc.scalar.copy(ob[:mw, :], pso[:mw, :])
                    nc.sync.dma_start(
                        out=out[b * S + m * 128 : b * S + m * 128 + mw, :],
                        in_=ob[:mw, :],
                    )

    nc.finalize()
    return nc


def _get_nc():
    if "nc" not in _cache:
        _cache["nc"] = _build()
    return _cache["nc"]


def _prep_shared(data, w):
    # layout-only host staging (no arithmetic)
    d0 = np.asarray(data, dtype=np.float32)[:, :, 0, :]  # [100, 166, 768]
    dT = np.transpose(d0, (0, 2, 1))  # [100, 768, 166]
    dT = (
        dT.reshape(NSNIP, EC, 128, P)
        .transpose(0, 2, 1, 3)
        .reshape(NSNIP * 128, EC * P)
    )
    dataT2 = np.ascontiguousarray(dT, dtype=np.float32)
    wT = np.asarray(w, dtype=np.float32).T  # [768, 11]
    w2 = np.ascontiguousarray(
        wT.reshape(EC, 128, W).transpose(1, 0, 2).reshape(128, EC * W),
        dtype=np.float32,
    )
    return dataT2, w2


def kernel(inputs, code_snippet_id, data, w, _trace=False):
    from concourse.bass_utils import run_bass_kernel_spmd

    nc = _get_nc()
    inputs = np.asarray(inputs, dtype=np.int32)
    code_snippet_id = np.asarray(code_snippet_id, dtype=np.int32)
    dataT2, w2 = _prep_shared(data, w)

    in_maps = []
    for ci in range(N_CORES):
        b0 = ci * BPC
        in_maps.append(
            {
                "inp": np.ascontiguousarray(
                    inputs[b0 : b0 + BPC].reshape(1, BPC * S)
                ),
                "snip": np.ascontiguousarray(
                    code_snippet_id[b0 : b0 + BPC].reshape(1, BPC)
                ),
                "dataT2": dataT2,
                "w2": w2,
            }
        )

    res = run_bass_kernel_spmd(
        nc, in_maps, core_ids=list(range(N_CORES)), trace=_trace
    )
    _cache["last_results"] = res
    out = np.concatenate(
        [res.results[i]["out"].reshape(BPC, S, E) for i in range(N_CORES)],
        axis=0,
    ).astype(np.float32)
    return out


# revision 18
# speedup vs baseline: 1.1798x; 1.1798x over previous
"""Trainium2 Bass kernel for windowed embedding lookup (nn_AttentionLayer).

Computation:
  out[b,s,e] = sum_k w[k,e] * data[snip_b, clip(inputs[b,s]+k-5, 0, 165), 0, e]

Strategy (data-parallel over batch, 2 batches per core on 8 cores):
  1. Per batch, gather the snippet's table slice T [166,768] (transposed
     [e,p] layout, staged by the host) via indirect DMA.
  2. Compute the 11-tap clip-padded convolution C[p,e] = sum_k w[k,e]*T[clip(p+k-5),e]
     on DVE/ACT (per-partition-scalar products + tensor adds).
  3. Transpose C back to [p,e] via TensorE.
  4. Gather rows out[s] = C[inputs[s]] as a one-hot matmul on TensorE.
  5. DMA the [1126,768] result straight to DRAM.

The host only does layout transforms (slice/transpose/reshape) and sharding;
all arithmetic runs on device.
"""

import sys

for _p in ("/opt/trn_rl_repo",):
    if _p not in sys.path:
        sys.path.insert(0, _p)

import numpy as np

N_CORES = 8
B = 16
BPC = B // N_CORES  # batches per core
S = 1126
E = 768
EC = 6  # number of 128-wide e chunks
P = 166  # table positions
PPAD = 176  # padded positions (5 on each side)
W = 11
NSNIP = 100
MTILES = (S + 127) // 128  # 9

_cache = {}


def _build(debug=False):
    import concourse.bass as bass
    import concourse.mybir as mybir
    import concourse.tile as tile
    from concourse import bacc
    from concourse.masks import make_identity

    f32 = mybir.dt.float32
    bf16 = mybir.dt.bfloat16
    i32 = mybir.dt.int32
    AOT = mybir.AluOpType

    nc = bacc.Bacc()
    dbg = {}
    if debug:
        dbg["t2"] = nc.declare_dram_parameter(
            "dbg_t2", [128, EC * PPAD], f32, isOutput=True
        )
        dbg["rows"] = nc.declare_dram_parameter(
            "dbg_rows", [128, 1], i32, isOutput=True
        )
        dbg["inpb"] = nc.declare_dram_parameter(
            "dbg_inpb", [128, S], f32, isOutput=True
        )
        dbg["oh0"] = nc.declare_dram_parameter(
            "dbg_oh0", [128, S], f32, isOutput=True
        )
        dbg["c2"] = nc.declare_dram_parameter(
            "dbg_c2", [128, EC * P], f32, isOutput=True
        )
        dbg["ccat0"] = nc.declare_dram_parameter(
            "dbg_ccat0", [128, E], f32, isOutput=True
        )

    inp = nc.declare_dram_parameter("inp", [1, BPC * S], i32, isOutput=False)
    snip = nc.declare_dram_parameter("snip", [1, BPC], i32, isOutput=False)
    # row (snip*128 + i) holds [c*166 + p] -> data[snip, p, 0, c*128 + i]
    dataT2 = nc.declare_dram_parameter(
        "dataT2", [NSNIP * 128, EC * P], f32, isOutput=False
    )
    # [i, c*11 + k] -> w[k, c*128 + i]
    w2 = nc.declare_dram_parameter("w2", [128, EC * W], f32, isOutput=False)
    out = nc.declare_dram_parameter("out", [BPC * S, E], f32, isOutput=True)

    with tile.TileContext(nc) as tc:
        with (
            tc.tile_pool(name="const", bufs=1) as constp,
            tc.tile_pool(name="work", bufs=2) as workp,
            tc.tile_pool(name="mm", bufs=2) as mmp,
            tc.tile_pool(name="ob", bufs=3) as obp,
            tc.tile_pool(name="psum_t", bufs=2, space="PSUM") as psumt,
            tc.tile_pool(name="psum_mm", bufs=2, space="PSUM") as psummm,
        ):
            ident = constp.tile([128, 128], bf16)
            make_identity(nc, ident[:])

            ones1 = constp.tile([1, 128], bf16)
            nc.gpsimd.memset(ones1[:], 1.0)

            iota_i = constp.tile([128, 1], i32)
            nc.gpsimd.iota(iota_i[:], [[1, 1]], base=0, channel_multiplier=1)
            iota_f = constp.tile([128, 1], f32)
            nc.vector.tensor_copy(iota_f[:], iota_i[:])
            iota_f_hi = constp.tile([128, 1], f32)
            nc.vector.tensor_scalar_add(iota_f_hi[:], iota_f[:], 128.0)

            w2t = constp.tile([128, EC * W], f32)
            nc.sync.dma_start(out=w2t[:], in_=w2[:])
            inpt = constp.tile([1, BPC * S], i32)
            nc.sync.dma_start(out=inpt[:], in_=inp[:])
            snipt = constp.tile([1, BPC], i32)
            nc.sync.dma_start(out=snipt[:], in_=snip[:])

            for b in range(BPC):
                # ---- table row indices: snip*128 + iota (per-partition)
                snipf = workp.tile([1, 1], bf16, tag="snipf")
                nc.vector.tensor_copy(snipf[:], snipt[0:1, b : b + 1])
                ps_sn = psumt.tile([128, 512], f32, tag="tp")
                nc.tensor.matmul(
                    out=ps_sn[:, :1],
                    lhsT=ones1[:, :],
                    rhs=snipf[:, :],
                    start=True,
                    stop=True,
                )
                rows_f = workp.tile([128, 1], f32, tag="rowsf")
                nc.vector.tensor_scalar(
                    rows_f[:], ps_sn[:, :1], 128.0, iota_f[:, :1],
                    AOT.mult, AOT.add,
                )
                rows2 = workp.tile([128, 1], i32, tag="rows2")
                nc.vector.tensor_copy(rows2[:], rows_f[:])
                if debug and b == 0:
                    nc.sync.dma_start(out=dbg["rows"][:], in_=rows2[:])

                # ---- gather the snippet table slice (contiguous dest), then
                # spread into the padded [e,p] layout
                t2raw = workp.tile([128, EC * P], f32, tag="t2raw")
                nc.gpsimd.indirect_dma_start(
                    out=t2raw[:, :],
                    out_offset=None,
                    in_=dataT2[:],
                    in_offset=bass.IndirectOffsetOnAxis(ap=rows2[:, :1], axis=0),
                )
                t2 = workp.tile([128, EC, PPAD], bf16, tag="t2")
                nc.vector.tensor_copy(
                    t2[:, :, 5 : 5 + P],
                    t2raw[:, :].rearrange("p (c q) -> p c q", q=P),
                )
                for c in range(EC):
                    nc.vector.tensor_copy(
                        t2[:, c, 0:5], t2[:, c, 5:6].to_broadcast([128, 5])
                    )
                    nc.vector.tensor_copy(
                        t2[:, c, 5 + P : PPAD],
                        t2[:, c, 4 + P : 5 + P].to_broadcast([128, 5]),
                    )
                if debug and b == 0:
                    nc.gpsimd.dma_start(out=dbg["t2"][:], in_=t2[:].rearrange("p c q -> p (c q)"))

                # ---- 11-tap conv: accA = sum_{k even}, accB = sum_{k odd}
                accA = workp.tile([128, EC, P], bf16, tag="accA")
                accB = workp.tile([128, EC, P], bf16, tag="accB")
                # k = 0 / k = 1 write straight into the accumulators
                for c in range(EC):
                    nc.vector.tensor_scalar(
                        accA[:, c, :], t2[:, c, 0:P],
                        w2t[:, c * W : c * W + 1], None, AOT.mult,
                    )
                    nc.scalar.activation(
                        accB[:, c, :], t2[:, c, 1 : 1 + P],
                        mybir.ActivationFunctionType.Copy,
                        scale=w2t[:, c * W + 1 : c * W + 2],
                    )
                for k in range(2, W):
                    prod = workp.tile([128, EC, P], bf16, tag=f"prod{k % 4}")
                    for c in range(EC):
                        if k % 2 == 0:
                            nc.vector.tensor_scalar(
                                prod[:, c, :], t2[:, c, k : k + P],
                                w2t[:, c * W + k : c * W + k + 1], None, AOT.mult,
                            )
                        else:
                            nc.scalar.activation(
                                prod[:, c, :], t2[:, c, k : k + P],
                                mybir.ActivationFunctionType.Copy,
                                scale=w2t[:, c * W + k : c * W + k + 1],
                            )
                    if k % 2 == 0:
                        nc.vector.tensor_add(accA[:], accA[:], prod[:])
                    else:
                        nc.gpsimd.tensor_add(accB[:], accB[:], prod[:])
                c2 = workp.tile([128, EC, P], bf16, tag="c2")
                nc.vector.tensor_add(c2[:], accA[:], accB[:])
                if debug and b == 0:
                    nc.gpsimd.dma_start(out=dbg["c2"][:], in_=c2[:].rearrange("p c q -> p (c q)"))

                # ---- transpose C2 [e,p] -> C [p,e] (2 partition chunks)
                ccat0 = mmp.tile([128, E], bf16, tag="c0")
                ccat1 = mmp.tile([128, E], bf16, tag="c1")
                nc.vector.memzero(ccat1[:])
                for c in range(EC):
                    ps0 = psumt.tile([128, 512], bf16, tag="tp")
                    nc.tensor.transpose(ps0[:, :128], c2[:, c, 0:128], ident[:])
                    nc.vector.tensor_copy(
                        ccat0[:, c * 128 : (c + 1) * 128], ps0[:, :128]
                    )
                    ps1 = psumt.tile([128, 512], bf16, tag="tp")
                    nc.tensor.transpose(ps1[:38, :128], c2[:, c, 128:P], ident[:])
                    nc.scalar.copy(
                        ccat1[:38, c * 128 : (c + 1) * 128], ps1[:38, :128]
                    )
                if debug and b == 0:
                    nc.gpsimd.dma_start(out=dbg["ccat0"][:], in_=ccat0[:])

                # ---- one-hot build (transposed): oh[p, s] = (inputs[s] == p)
                # replicate the inputs row across partitions via ones-matmul
                inpr_f = workp.tile([1, S], bf16, tag="inprf")
                nc.vector.tensor_copy(
                    inpr_f[:], inpt[0:1, b * S : (b + 1) * S]
                )
                inpb_f = workp.tile([128, S], bf16, tag="inpbf")
                for n0 in range(0, S, 512):
                    nw = min(512, S - n0)
                    ps_in = psumt.tile([128, 512], f32, tag="tp")
                    nc.tensor.matmul(
                        out=ps_in[:, :nw],
                        lhsT=ones1[:, :],
                        rhs=inpr_f[:, n0 : n0 + nw],
                        start=True,
                        stop=True,
                    )
                    nc.vector.tensor_copy(
                        inpb_f[:, n0 : n0 + nw], ps_in[:, :nw]
                    )
                oh0 = mmp.tile([128, S], bf16, tag="oh0")
                oh1 = mmp.tile([128, S], bf16, tag="oh1")
                nc.vector.tensor_scalar(
                    oh0[:], inpb_f[:], iota_f[:, :1], None, AOT.is_equal
                )
                nc.vector.tensor_scalar(
                    oh1[:], inpb_f[:], iota_f_hi[:, :1], None, AOT.is_equal
                )
                if debug and b == 0:
                    nc.gpsimd.dma_start(out=dbg["inpb"][:], in_=inpb_f[:])
                    nc.gpsimd.dma_start(out=dbg["oh0"][:], in_=oh0[:])

                # ---- gather matmul: out[s, e] = sum_p oh[p, s] * C[p, e]
                for m in range(MTILES):
                    mw = min(128, S - m * 128)
                    pso = psummm.tile([128, E], f32, tag="po")
                    for oh, cc, st in ((oh0, ccat0, True), (oh1, ccat1, False)):
                        for n0, nw in ((0, 512), (512, 256)):
                            nc.tensor.matmul(
                                out=pso[:mw, n0 : n0 + nw],
                                lhsT=oh[:, m * 128 : m * 128 + mw],
                                rhs=cc[:, n0 : n0 + nw],
                                start=st,
                                stop=not st,
                            )
                    ob = obp.tile([128, E], f32, tag="ob")
                    if m % 2 == 0:
                        nc.vector.tensor_copy(ob[:mw, :], pso[:mw, :])
                    else:
                        nc.scalar.copy(ob[:mw, :], pso[:mw, :])
                    nc.sync.dma_start(
                        out=out[b * S + m * 128 : b * S + m * 128 + mw, :],
                        in_=ob[:mw, :],
                    )

    nc.finalize()
    return nc


def _get_nc():
    if "nc" not in _cache:
        _cache["nc"] = _build()
    return _cache["nc"]


def _prep_shared(data, w):
    # layout-only host staging (no arithmetic)
    d0 = np.asarray(data, dtype=np.float32)[:, :, 0, :]  # [100, 166, 768]
    dT = np.transpose(d0, (0, 2, 1))  # [100, 768, 166]
    dT = (
        dT.reshape(NSNIP, EC, 128, P)
        .transpose(0, 2, 1, 3)
        .reshape(NSNIP * 128, EC * P)
    )
    dataT2 = np.ascontiguousarray(dT, dtype=np.float32)
    wT = np.asarray(w, dtype=np.float32).T  # [768, 11]
    w2 = np.ascontiguousarray(
        wT.reshape(EC, 128, W).transpose(1, 0, 2).reshape(128, EC * W),
        dtype=np.float32,
    )
    return dataT2, w2


def kernel(inputs, code_snippet_id, data, w, _trace=False):
    from concourse.bass_utils import run_bass_kernel_spmd

    nc = _get_nc()
    inputs = np.asarray(inputs, dtype=np.int32)
    code_snippet_id = np.asarray(code_snippet_id, dtype=np.int32)
    dataT2, w2 = _prep_shared(data, w)

    in_maps = []
    for ci in range(N_CORES):
        b0 = ci * BPC
        in_maps.append(
            {
                "inp": np.ascontiguousarray(
                    inputs[b0 : b0 + BPC].reshape(1, BPC * S)
                ),
                "snip": np.ascontiguousarray(
                    code_snippet_id[b0 : b0 + BPC].reshape(1, BPC)
                ),
                "dataT2": dataT2,
                "w2": w2,
            }
        )

    res = run_bass_kernel_spmd(
        nc, in_maps, core_ids=list(range(N_CORES)), trace=_trace
    )
    _cache["last_results"] = res
    out = np.concatenate(
        [res.results[i]["out"].reshape(BPC, S, E) for i in range(N_CORES)],
        axis=0,
    ).astype(np.float32)
    return out


# revision 21
# speedup vs baseline: 1.6400x; 1.3900x over previous
"""Trainium2 Bass kernel for windowed embedding lookup (nn_AttentionLayer).

Computation:
  out[b,s,e] = sum_k w[k,e] * data[snip_b, clip(inputs[b,s]+k-5, 0, 165), 0, e]

Strategy (data-parallel over batch, 2 batches per core on 8 cores):
  1. Per batch, gather the snippet's table slice T [166,768] (transposed
     [e,p] layout, staged by the host) via indirect DMA.
  2. Compute the 11-tap clip-padded convolution C[p,e] = sum_k w[k,e]*T[clip(p+k-5),e]
     on DVE/ACT (per-partition-scalar products + tensor adds).
  3. Transpose C back to [p,e] via TensorE.
  4. Gather rows out[s] = C[inputs[s]] as a one-hot matmul on TensorE.
  5. DMA the [1126,768] result straight to DRAM.

The host only does layout transforms (slice/transpose/reshape) and sharding;
all arithmetic runs on device.
"""

import sys

for _p in ("/opt/trn_rl_repo",):
    if _p not in sys.path:
        sys.path.insert(0, _p)

import numpy as np

N_CORES = 8
B = 16
BPC = B // N_CORES  # batches per core
S = 1126
E = 768
EC = 6  # number of 128-wide e chunks
P = 166  # table positions
PPAD = 176  # padded positions (5 on each side)
W = 11
NSNIP = 100
MTILES = (S + 127) // 128  # 9

_cache = {}


def _build(debug=False):
    import concourse.bass as bass
    import concourse.mybir as mybir
    import concourse.tile as tile
    from concourse import bacc
    from concourse.masks import make_identity

    f32 = mybir.dt.float32
    bf16 = mybir.dt.bfloat16
    i32 = mybir.dt.int32
    AOT = mybir.AluOpType

    nc = bacc.Bacc()
    dbg = {}
    if debug:
        dbg["t2"] = nc.declare_dram_parameter(
            "dbg_t2", [128, EC * PPAD], f32, isOutput=True
        )
        dbg["rows"] = nc.declare_dram_parameter(
            "dbg_rows", [128, 1], i32, isOutput=True
        )
        dbg["inpb"] = nc.declare_dram_parameter(
            "dbg_inpb", [128, S], f32, isOutput=True
        )
        dbg["oh0"] = nc.declare_dram_parameter(
            "dbg_oh0", [128, S], f32, isOutput=True
        )
        dbg["c2"] = nc.declare_dram_parameter(
            "dbg_c2", [128, EC * P], f32, isOutput=True
        )
        dbg["ccat0"] = nc.declare_dram_parameter(
            "dbg_ccat0", [128, E], f32, isOutput=True
        )

    inp = nc.declare_dram_parameter("inp", [1, BPC * S], i32, isOutput=False)
    snip = nc.declare_dram_parameter("snip", [1, BPC], i32, isOutput=False)
    # row (snip*128 + i) holds [c*166 + p] -> data[snip, p, 0, c*128 + i]
    dataT2 = nc.declare_dram_parameter(
        "dataT2", [NSNIP * 128, EC * P], f32, isOutput=False
    )
    # diagonal weight matrices: [i, (c*11+k)*128 + j] = w[k, c*128+i] iff i==j
    diagw = nc.declare_dram_parameter(
        "diagw", [128, EC * W * 128], f32, isOutput=False
    )
    out = nc.declare_dram_parameter("out", [BPC * S, E], f32, isOutput=True)

    with tile.TileContext(nc) as tc:
        with (
            tc.tile_pool(name="const", bufs=1) as constp,
            tc.tile_pool(name="work", bufs=2) as workp,
            tc.tile_pool(name="mm", bufs=2) as mmp,
            tc.tile_pool(name="ob", bufs=3) as obp,
            tc.tile_pool(name="psum_t", bufs=2, space="PSUM") as psumt,
            tc.tile_pool(name="psum_c", bufs=2, space="PSUM") as psumc,
            tc.tile_pool(name="psum_mm", bufs=2, space="PSUM") as psummm,
        ):
            ident = constp.tile([128, 128], bf16)
            make_identity(nc, ident[:])

            ones1 = constp.tile([1, 128], bf16)
            nc.gpsimd.memset(ones1[:], 1.0)

            iota_i = constp.tile([128, 1], i32)
            nc.gpsimd.iota(iota_i[:], [[1, 1]], base=0, channel_multiplier=1)
            iota_f = constp.tile([128, 1], f32)
            nc.vector.tensor_copy(iota_f[:], iota_i[:])
            iota_f_hi = constp.tile([128, 1], f32)
            nc.vector.tensor_scalar_add(iota_f_hi[:], iota_f[:], 128.0)

            diagb = constp.tile([128, EC * W * 128], bf16)
            nc.gpsimd.dma_start(out=diagb[:], in_=diagw[:])
            inpt = constp.tile([1, BPC * S], i32)
            nc.sync.dma_start(out=inpt[:], in_=inp[:])
            snipt = constp.tile([1, BPC], i32)
            nc.sync.dma_start(out=snipt[:], in_=snip[:])

            for b in range(BPC):
                # ---- table row indices: snip*128 + iota (per-partition)
                snipf = workp.tile([1, 1], bf16, tag="snipf")
                nc.vector.tensor_copy(snipf[:], snipt[0:1, b : b + 1])
                ps_sn = psumt.tile([128, 512], f32, tag="tp")
                nc.tensor.matmul(
                    out=ps_sn[:, :1],
                    lhsT=ones1[:, :],
                    rhs=snipf[:, :],
                    start=True,
                    stop=True,
                )
                rows_f = workp.tile([128, 1], f32, tag="rowsf")
                nc.vector.tensor_scalar(
                    rows_f[:], ps_sn[:, :1], 128.0, iota_f[:, :1],
                    AOT.mult, AOT.add,
                )
                rows2 = workp.tile([128, 1], i32, tag="rows2")
                nc.vector.tensor_copy(rows2[:], rows_f[:])
                if debug and b == 0:
                    nc.sync.dma_start(out=dbg["rows"][:], in_=rows2[:])

                # ---- gather the snippet table slice (contiguous dest), then
                # spread into the padded [e,p] layout
                t2raw = workp.tile([128, EC * P], f32, tag="t2raw")
                nc.gpsimd.indirect_dma_start(
                    out=t2raw[:, :],
                    out_offset=None,
                    in_=dataT2[:],
                    in_offset=bass.IndirectOffsetOnAxis(ap=rows2[:, :1], axis=0),
                )
                t2 = workp.tile([128, EC, PPAD], bf16, tag="t2")
                nc.vector.tensor_copy(
                    t2[:, :, 5 : 5 + P],
                    t2raw[:, :].rearrange("p (c q) -> p c q", q=P),
                )
                for c in range(EC):
                    nc.vector.tensor_copy(
                        t2[:, c, 0:5], t2[:, c, 5:6].to_broadcast([128, 5])
                    )
                    nc.vector.tensor_copy(
                        t2[:, c, 5 + P : PPAD],
                        t2[:, c, 4 + P : 5 + P].to_broadcast([128, 5]),
                    )
                if debug and b == 0:
                    nc.gpsimd.dma_start(out=dbg["t2"][:], in_=t2[:].rearrange("p c q -> p (c q)"))

                # ---- 11-tap conv on TensorE: for each e-chunk, accumulate
                # 11 diag(w_k) @ t2[:, c, k:k+P] matmuls in PSUM
                c2 = workp.tile([128, EC, P], bf16, tag="c2")
                for c in range(EC):
                    psc = psumc.tile([128, P], f32, tag="pc")
                    for k in range(W):
                        nc.tensor.matmul(
                            out=psc[:, :],
                            lhsT=diagb[:, (c * W + k) * 128 : (c * W + k + 1) * 128],
                            rhs=t2[:, c, k : k + P],
                            start=(k == 0),
                            stop=(k == W - 1),
                        )
                    if c % 2 == 0:
                        nc.vector.tensor_copy(c2[:, c, :], psc[:, :])
                    else:
                        nc.scalar.copy(c2[:, c, :], psc[:, :])
                if debug and b == 0:
                    nc.gpsimd.dma_start(out=dbg["c2"][:], in_=c2[:].rearrange("p c q -> p (c q)"))

                # ---- transpose C2 [e,p] -> C [p,e] (2 partition chunks)
                ccat0 = mmp.tile([128, E], bf16, tag="c0")
                ccat1 = mmp.tile([128, E], bf16, tag="c1")
                nc.vector.memzero(ccat1[:])
                for c in range(EC):
                    ps0 = psumt.tile([128, 512], bf16, tag="tp")
                    nc.tensor.transpose(ps0[:, :128], c2[:, c, 0:128], ident[:])
                    nc.vector.tensor_copy(
                        ccat0[:, c * 128 : (c + 1) * 128], ps0[:, :128]
                    )
                    ps1 = psumt.tile([128, 512], bf16, tag="tp")
                    nc.tensor.transpose(ps1[:38, :128], c2[:, c, 128:P], ident[:])
                    nc.scalar.copy(
                        ccat1[:38, c * 128 : (c + 1) * 128], ps1[:38, :128]
                    )
                if debug and b == 0:
                    nc.gpsimd.dma_start(out=dbg["ccat0"][:], in_=ccat0[:])

                # ---- one-hot build (transposed): oh[p, s] = (inputs[s] == p)
                # replicate the inputs row across partitions via ones-matmul
                inpr_f = workp.tile([1, S], bf16, tag="inprf")
                nc.vector.tensor_copy(
                    inpr_f[:], inpt[0:1, b * S : (b + 1) * S]
                )
                inpb_f = workp.tile([128, S], bf16, tag="inpbf")
                for n0 in range(0, S, 512):
                    nw = min(512, S - n0)
                    ps_in = psumt.tile([128, 512], f32, tag="tp")
                    nc.tensor.matmul(
                        out=ps_in[:, :nw],
                        lhsT=ones1[:, :],
                        rhs=inpr_f[:, n0 : n0 + nw],
                        start=True,
                        stop=True,
                    )
                    nc.vector.tensor_copy(
                        inpb_f[:, n0 : n0 + nw], ps_in[:, :nw]
                    )
                oh0 = mmp.tile([128, S], bf16, tag="oh0")
                oh1 = mmp.tile([128, S], bf16, tag="oh1")
                nc.vector.tensor_scalar(
                    oh0[:], inpb_f[:], iota_f[:, :1], None, AOT.is_equal
                )
                nc.vector.tensor_scalar(
                    oh1[:], inpb_f[:], iota_f_hi[:, :1], None, AOT.is_equal
                )
                if debug and b == 0:
                    nc.gpsimd.dma_start(out=dbg["inpb"][:], in_=inpb_f[:])
                    nc.gpsimd.dma_start(out=dbg["oh0"][:], in_=oh0[:])

                # ---- gather matmul: out[s, e] = sum_p oh[p, s] * C[p, e]
                for m in range(MTILES):
                    mw = min(128, S - m * 128)
                    pso = psummm.tile([128, E], f32, tag="po")
                    for oh, cc, st in ((oh0, ccat0, True), (oh1, ccat1, False)):
                        for n0, nw in ((0, 512), (512, 256)):
                            nc.tensor.matmul(
                                out=pso[:mw, n0 : n0 + nw],
                                lhsT=oh[:, m * 128 : m * 128 + mw],
                                rhs=cc[:, n0 : n0 + nw],
                                start=st,
                                stop=not st,
                            )
                    ob = obp.tile([128, E], f32, tag="ob")
                    if m % 2 == 0:
                        nc.vector.tensor_copy(ob[:mw, :], pso[:mw, :])
                    else:
                        nc.scalar.copy(ob[:mw, :], pso[:mw, :])
                    nc.sync.dma_start(
                        out=out[b * S + m * 128 : b * S + m * 128 + mw, :],
                        in_=ob[:mw, :],
                    )

    nc.finalize()
    return nc


def _get_nc():
    if "nc" not in _cache:
        _cache["nc"] = _build()
    return _cache["nc"]


def _prep_shared(data, w):
    # layout-only host staging (no arithmetic)
    d0 = np.asarray(data, dtype=np.float32)[:, :, 0, :]  # [100, 166, 768]
    dT = np.transpose(d0, (0, 2, 1))  # [100, 768, 166]
    dT = (
        dT.reshape(NSNIP, EC, 128, P)
        .transpose(0, 2, 1, 3)
        .reshape(NSNIP * 128, EC * P)
    )
    dataT2 = np.ascontiguousarray(dT, dtype=np.float32)
    wT = np.asarray(w, dtype=np.float32).T  # [768, 11]
    w2 = wT.reshape(EC, 128, W).transpose(1, 0, 2)  # [128, EC, W]
    diagw = np.zeros((128, EC * W, 128), dtype=np.float32)
    ii = np.arange(128)
    diagw[ii, :, ii] = w2.reshape(128, EC * W)
    diagw = np.ascontiguousarray(diagw.reshape(128, EC * W * 128))
    return dataT2, diagw


def kernel(inputs, code_snippet_id, data, w, _trace=False):
    from concourse.bass_utils import run_bass_kernel_spmd

    nc = _get_nc()
    inputs = np.asarray(inputs, dtype=np.int32)
    code_snippet_id = np.asarray(code_snippet_id, dtype=np.int32)
    dataT2, diagw = _prep_shared(data, w)

    in_maps = []
    for ci in range(N_CORES):
        b0 = ci * BPC
        in_maps.append(
            {
                "inp": np.ascontiguousarray(
                    inputs[b0 : b0 + BPC].reshape(1, BPC * S)
                ),
                "snip": np.ascontiguousarray(
                    code_snippet_id[b0 : b0 + BPC].reshape(1, BPC)
                ),
                "dataT2": dataT2,
                "diagw": diagw,
            }
        )

    res = run_bass_kernel_spmd(
        nc, in_maps, core_ids=list(range(N_CORES)), trace=_trace
    )
    _cache["last_results"] = res
    out = np.concatenate(
        [res.results[i]["out"].reshape(BPC, S, E) for i in range(N_CORES)],
        axis=0,
    ).astype(np.float32)
    return out


# revision 23
# speedup vs baseline: 1.8138x; 1.1060x over previous
"""Trainium2 Bass kernel for windowed embedding lookup (nn_AttentionLayer).

Computation:
  out[b,s,e] = sum_k w[k,e] * data[snip_b, clip(inputs[b,s]+k-5, 0, 165), 0, e]

Strategy (data-parallel over batch, 2 batches per core on 8 cores):
  1. Per batch, gather the snippet's table slice T [166,768] (transposed
     [e,p] layout, staged by the host) via indirect DMA.
  2. Compute the 11-tap clip-padded convolution C[p,e] = sum_k w[k,e]*T[clip(p+k-5),e]
     on DVE/ACT (per-partition-scalar products + tensor adds).
  3. Transpose C back to [p,e] via TensorE.
  4. Gather rows out[s] = C[inputs[s]] as a one-hot matmul on TensorE.
  5. DMA the [1126,768] result straight to DRAM.

The host only does layout transforms (slice/transpose/reshape) and sharding;
all arithmetic runs on device.
"""

import sys

for _p in ("/opt/trn_rl_repo",):
    if _p not in sys.path:
        sys.path.insert(0, _p)

import numpy as np

N_CORES = 8
B = 16
BPC = B // N_CORES  # batches per core
S = 1126
E = 768
EC = 6  # number of 128-wide e chunks
P = 166  # table positions
PPAD = 176  # padded positions (5 on each side)
W = 11
NSNIP = 100
MTILES = (S + 127) // 128  # 9

_cache = {}


def _build(debug=False):
    import concourse.bass as bass
    import concourse.mybir as mybir
    import concourse.tile as tile
    from concourse import bacc
    from concourse.masks import make_identity

    f32 = mybir.dt.float32
    bf16 = mybir.dt.bfloat16
    i32 = mybir.dt.int32
    AOT = mybir.AluOpType

    nc = bacc.Bacc()
    dbg = {}
    if debug:
        dbg["t2"] = nc.declare_dram_parameter(
            "dbg_t2", [128, EC * PPAD], f32, isOutput=True
        )
        dbg["rows"] = nc.declare_dram_parameter(
            "dbg_rows", [128, 1], i32, isOutput=True
        )
        dbg["inpb"] = nc.declare_dram_parameter(
            "dbg_inpb", [128, S], f32, isOutput=True
        )
        dbg["oh0"] = nc.declare_dram_parameter(
            "dbg_oh0", [128, S], f32, isOutput=True
        )
        dbg["c2"] = nc.declare_dram_parameter(
            "dbg_c2", [128, EC * P], f32, isOutput=True
        )
        dbg["ccat0"] = nc.declare_dram_parameter(
            "dbg_ccat0", [128, E], f32, isOutput=True
        )

    inp = nc.declare_dram_parameter("inp", [1, BPC * S], i32, isOutput=False)
    snip = nc.declare_dram_parameter("snip", [1, BPC], i32, isOutput=False)
    # row (snip*128 + i) holds [c*166 + p] -> data[snip, p, 0, c*128 + i]
    dataT2 = nc.declare_dram_parameter(
        "dataT2", [NSNIP * 128, EC * P], f32, isOutput=False
    )
    # [i, c*11 + k] -> w[k, c*128 + i]
    w2 = nc.declare_dram_parameter("w2", [128, EC * W], f32, isOutput=False)
    out = nc.declare_dram_parameter("out", [BPC * S, E], f32, isOutput=True)

    with tile.TileContext(nc) as tc:
        with (
            tc.tile_pool(name="const", bufs=1) as constp,
            tc.tile_pool(name="work", bufs=2) as workp,
            tc.tile_pool(name="mm", bufs=2) as mmp,
            tc.tile_pool(name="ob", bufs=3) as obp,
            tc.tile_pool(name="psum_t", bufs=2, space="PSUM") as psumt,
            tc.tile_pool(name="psum_c", bufs=2, space="PSUM") as psumc,
            tc.tile_pool(name="psum_mm", bufs=2, space="PSUM") as psummm,
        ):
            ones1 = constp.tile([1, 128], bf16)
            nc.gpsimd.memset(ones1[:], 1.0)

            iota_i = constp.tile([128, 1], i32)
            nc.gpsimd.iota(iota_i[:], [[1, 1]], base=0, channel_multiplier=1)
            iota_f = constp.tile([128, 1], f32)
            nc.vector.tensor_copy(iota_f[:], iota_i[:])
            iota_f_hi = constp.tile([128, 1], f32)
            nc.vector.tensor_scalar_add(iota_f_hi[:], iota_f[:], 128.0)

            w2b = constp.tile([128, EC * W], bf16)
            nc.gpsimd.dma_start(out=w2b[:], in_=w2[:])
            diagb = constp.tile([128, EC * W, 128], bf16)
            nc.gpsimd.affine_select(
                out=diagb[:],
                in_=w2b[:, :, None].to_broadcast([128, EC * W, 128]),
                pattern=[[0, EC * W], [-1, 128]],
                compare_op=AOT.is_equal,
                fill=0.0,
                base=0,
                channel_multiplier=1,
            )
            inpt = constp.tile([1, BPC * S], i32)
            nc.sync.dma_start(out=inpt[:], in_=inp[:])
            snipt = constp.tile([1, BPC], i32)
            nc.sync.dma_start(out=snipt[:], in_=snip[:])

            for b in range(BPC):
                # ---- table row indices: snip*128 + iota (per-partition)
                snipf = workp.tile([1, 1], bf16, tag="snipf")
                nc.vector.tensor_copy(snipf[:], snipt[0:1, b : b + 1])
                ps_sn = psumt.tile([128, 512], f32, tag="tp")
                nc.tensor.matmul(
                    out=ps_sn[:, :1],
                    lhsT=ones1[:, :],
                    rhs=snipf[:, :],
                    start=True,
                    stop=True,
                )
                rows_f = workp.tile([128, 1], f32, tag="rowsf")
                nc.vector.tensor_scalar(
                    rows_f[:], ps_sn[:, :1], 128.0, iota_f[:, :1],
                    AOT.mult, AOT.add,
                )
                rows2 = workp.tile([128, 1], i32, tag="rows2")
                nc.vector.tensor_copy(rows2[:], rows_f[:])
                if debug and b == 0:
                    nc.sync.dma_start(out=dbg["rows"][:], in_=rows2[:])

                # ---- gather the snippet table slice (contiguous dest), then
                # spread into the padded [e,p] layout
                t2raw = workp.tile([128, EC * P], f32, tag="t2raw")
                nc.gpsimd.indirect_dma_start(
                    out=t2raw[:, :],
                    out_offset=None,
                    in_=dataT2[:],
                    in_offset=bass.IndirectOffsetOnAxis(ap=rows2[:, :1], axis=0),
                )
                t2 = workp.tile([128, EC, PPAD], bf16, tag="t2")
                nc.vector.tensor_copy(
                    t2[:, :, 5 : 5 + P],
                    t2raw[:, :].rearrange("p (c q) -> p c q", q=P),
                )
                for c in range(EC):
                    nc.vector.tensor_copy(
                        t2[:, c, 0:5], t2[:, c, 5:6].to_broadcast([128, 5])
                    )
                    nc.vector.tensor_copy(
                        t2[:, c, 5 + P : PPAD],
                        t2[:, c, 4 + P : 5 + P].to_broadcast([128, 5]),
                    )
                if debug and b == 0:
                    nc.gpsimd.dma_start(out=dbg["t2"][:], in_=t2[:].rearrange("p c q -> p (c q)"))

                # ---- 11-tap conv on TensorE, output directly in [p, e]:
                # out[p', e'] = sum_i t2[i, c, off+p'+k] * diag_ck[i, e']
                ccat0 = mmp.tile([128, E], bf16, tag="c0")
                ccat1 = mmp.tile([128, E], bf16, tag="c1")
                nc.vector.memzero(ccat1[:])
                for c in range(EC):
                    for pc, pcw in ((0, 128), (1, P - 128)):
                        psc = psumc.tile([128, 128], f32, tag="pc")
                        for k in range(W):
                            nc.tensor.matmul(
                                out=psc[:pcw, :],
                                lhsT=t2[:, c, k + pc * 128 : k + pc * 128 + pcw],
                                rhs=diagb[:, c * W + k, :],
                                start=(k == 0),
                                stop=(k == W - 1),
                            )
                        cdst = (ccat0 if pc == 0 else ccat1)
                        if (c + pc) % 2 == 0:
                            nc.vector.tensor_copy(
                                cdst[:pcw, c * 128 : (c + 1) * 128], psc[:pcw, :]
                            )
                        else:
                            nc.scalar.copy(
                                cdst[:pcw, c * 128 : (c + 1) * 128], psc[:pcw, :]
                            )
                if debug and b == 0:
                    nc.gpsimd.dma_start(out=dbg["ccat0"][:], in_=ccat0[:])

                # ---- one-hot build (transposed): oh[p, s] = (inputs[s] == p)
                # replicate the inputs row across partitions via ones-matmul
                inpr_f = workp.tile([1, S], bf16, tag="inprf")
                nc.vector.tensor_copy(
                    inpr_f[:], inpt[0:1, b * S : (b + 1) * S]
                )
                inpb_f = workp.tile([128, S], bf16, tag="inpbf")
                for n0 in range(0, S, 512):
                    nw = min(512, S - n0)
                    ps_in = psumt.tile([128, 512], f32, tag="tp")
                    nc.tensor.matmul(
                        out=ps_in[:, :nw],
                        lhsT=ones1[:, :],
                        rhs=inpr_f[:, n0 : n0 + nw],
                        start=True,
                        stop=True,
                    )
                    nc.vector.tensor_copy(
                        inpb_f[:, n0 : n0 + nw], ps_in[:, :nw]
                    )
                oh0 = mmp.tile([128, S], bf16, tag="oh0")
                oh1 = mmp.tile([128, S], bf16, tag="oh1")
                nc.vector.tensor_scalar(
                    oh0[:], inpb_f[:], iota_f[:, :1], None, AOT.is_equal
                )
                nc.vector.tensor_scalar(
                    oh1[:], inpb_f[:], iota_f_hi[:, :1], None, AOT.is_equal
                )
                if debug and b == 0:
                    nc.gpsimd.dma_start(out=dbg["inpb"][:], in_=inpb_f[:])
                    nc.gpsimd.dma_start(out=dbg["oh0"][:], in_=oh0[:])

                # ---- gather matmul: out[s, e] = sum_p oh[p, s] * C[p, e]
                for m in range(MTILES):
                    mw = min(128, S - m * 128)
                    pso = psummm.tile([128, E], f32, tag="po")
                    for oh, cc, st in ((oh0, ccat0, True), (oh1, ccat1, False)):
                        for n0, nw in ((0, 512), (512, 256)):
                            nc.tensor.matmul(
                                out=pso[:mw, n0 : n0 + nw],
                                lhsT=oh[:, m * 128 : m * 128 + mw],
                                rhs=cc[:, n0 : n0 + nw],
                                start=st,
                                stop=not st,
                            )
                    ob = obp.tile([128, E], f32, tag="ob")
                    if m % 2 == 0:
                        nc.vector.tensor_copy(ob[:mw, :], pso[:mw, :])
                    else:
                        nc.scalar.copy(ob[:mw, :], pso[:mw, :])
                    nc.sync.dma_start(
                        out=out[b * S + m * 128 : b * S + m * 128 + mw, :],
                        in_=ob[:mw, :],
                    )

    nc.finalize()
    return nc


def _get_nc():
    if "nc" not in _cache:
        _cache["nc"] = _build()
    return _cache["nc"]


def _prep_shared(data, w):
    # layout-only host staging (no arithmetic)
    d0 = np.asarray(data, dtype=np.float32)[:, :, 0, :]  # [100, 166, 768]
    dT = np.transpose(d0, (0, 2, 1))  # [100, 768, 166]
    dT = (
        dT.reshape(NSNIP, EC, 128, P)
        .transpose(0, 2, 1, 3)
        .reshape(NSNIP * 128, EC * P)
    )
    dataT2 = np.ascontiguousarray(dT, dtype=np.float32)
    wT = np.asarray(w, dtype=np.float32).T  # [768, 11]
    w2 = np.ascontiguousarray(
        wT.reshape(EC, 128, W).transpose(1, 0, 2).reshape(128, EC * W),
        dtype=np.float32,
    )
    return dataT2, w2


def kernel(inputs, code_snippet_id, data, w, _trace=False):
    from concourse.bass_utils import run_bass_kernel_spmd

    nc = _get_nc()
    inputs = np.asarray(inputs, dtype=np.int32)
    code_snippet_id = np.asarray(code_snippet_id, dtype=np.int32)
    dataT2, w2 = _prep_shared(data, w)

    in_maps = []
    for ci in range(N_CORES):
        b0 = ci * BPC
        in_maps.append(
            {
                "inp": np.ascontiguousarray(
                    inputs[b0 : b0 + BPC].reshape(1, BPC * S)
                ),
                "snip": np.ascontiguousarray(
                    code_snippet_id[b0 : b0 + BPC].reshape(1, BPC)
                ),
                "dataT2": dataT2,
                "w2": w2,
            }
        )

    res = run_bass_kernel_spmd(
        nc, in_maps, core_ids=list(range(N_CORES)), trace=_trace
    )
    _cache["last_results"] = res
    out = np.concatenate(
        [res.results[i]["out"].reshape(BPC, S, E) for i in range(N_CORES)],
        axis=0,
    ).astype(np.float32)
    return out


# revision 24
# speedup vs baseline: 1.9422x; 1.0708x over previous
"""Trainium2 Bass kernel for windowed embedding lookup (nn_AttentionLayer).

Computation:
  out[b,s,e] = sum_k w[k,e] * data[snip_b, clip(inputs[b,s]+k-5, 0, 165), 0, e]

Strategy (data-parallel over batch, 2 batches per core on 8 cores):
  1. Per batch, gather the snippet's table slice T [166,768] (transposed
     [e,p] layout, staged by the host) via indirect DMA.
  2. Compute the 11-tap clip-padded convolution C[p,e] = sum_k w[k,e]*T[clip(p+k-5),e]
     on DVE/ACT (per-partition-scalar products + tensor adds).
  3. Transpose C back to [p,e] via TensorE.
  4. Gather rows out[s] = C[inputs[s]] as a one-hot matmul on TensorE.
  5. DMA the [1126,768] result straight to DRAM.

The host only does layout transforms (slice/transpose/reshape) and sharding;
all arithmetic runs on device.
"""

import sys

for _p in ("/opt/trn_rl_repo",):
    if _p not in sys.path:
        sys.path.insert(0, _p)

import numpy as np

N_CORES = 8
B = 16
BPC = B // N_CORES  # batches per core
S = 1126
E = 768
EC = 6  # number of 128-wide e chunks
P = 166  # table positions
PPAD = 176  # padded positions (5 on each side)
W = 11
NSNIP = 100
MTILES = (S + 127) // 128  # 9

_cache = {}


def _build(debug=False):
    import concourse.bass as bass
    import concourse.mybir as mybir
    import concourse.tile as tile
    from concourse import bacc
    from concourse.masks import make_identity

    f32 = mybir.dt.float32
    bf16 = mybir.dt.bfloat16
    i32 = mybir.dt.int32
    AOT = mybir.AluOpType

    nc = bacc.Bacc()
    dbg = {}
    if debug:
        dbg["t2"] = nc.declare_dram_parameter(
            "dbg_t2", [128, EC * PPAD], f32, isOutput=True
        )
        dbg["rows"] = nc.declare_dram_parameter(
            "dbg_rows", [128, 1], i32, isOutput=True
        )
        dbg["inpb"] = nc.declare_dram_parameter(
            "dbg_inpb", [128, S], f32, isOutput=True
        )
        dbg["oh0"] = nc.declare_dram_parameter(
            "dbg_oh0", [128, S], f32, isOutput=True
        )
        dbg["c2"] = nc.declare_dram_parameter(
            "dbg_c2", [128, EC * P], f32, isOutput=True
        )
        dbg["ccat0"] = nc.declare_dram_parameter(
            "dbg_ccat0", [128, E], f32, isOutput=True
        )

    inp = nc.declare_dram_parameter("inp", [1, BPC * S], i32, isOutput=False)
    snip = nc.declare_dram_parameter("snip", [1, BPC], i32, isOutput=False)
    # row (snip*128 + i) holds [c*166 + p] -> data[snip, p, 0, c*128 + i]
    dataT2 = nc.declare_dram_parameter(
        "dataT2", [NSNIP * 128, EC * P], f32, isOutput=False
    )
    # [i, c*11 + k] -> w[k, c*128 + i]
    w2 = nc.declare_dram_parameter("w2", [128, EC * W], f32, isOutput=False)
    out = nc.declare_dram_parameter("out", [BPC * S, E], f32, isOutput=True)

    with tile.TileContext(nc) as tc:
        with (
            tc.tile_pool(name="const", bufs=1) as constp,
            tc.tile_pool(name="work", bufs=2) as workp,
            tc.tile_pool(name="mm", bufs=2) as mmp,
            tc.tile_pool(name="ob", bufs=3) as obp,
            tc.tile_pool(name="psum_t", bufs=2, space="PSUM") as psumt,
            tc.tile_pool(name="psum_c", bufs=2, space="PSUM") as psumc,
            tc.tile_pool(name="psum_mm", bufs=2, space="PSUM") as psummm,
        ):
            ones1 = constp.tile([1, 128], bf16)
            nc.vector.memset(ones1[:], 1.0)

            iota_i = constp.tile([128, 1], i32)
            nc.gpsimd.iota(iota_i[:], [[1, 1]], base=0, channel_multiplier=1)
            iota_f = constp.tile([128, 1], f32)
            nc.vector.tensor_copy(iota_f[:], iota_i[:])
            iota_f_hi = constp.tile([128, 1], f32)
            nc.vector.tensor_scalar_add(iota_f_hi[:], iota_f[:], 128.0)

            w2b = constp.tile([128, EC * W], bf16)
            nc.gpsimd.dma_start(out=w2b[:], in_=w2[:])
            inpt = constp.tile([1, BPC * S], i32)
            nc.sync.dma_start(out=inpt[:], in_=inp[:])
            snipt = constp.tile([1, BPC], i32)
            nc.sync.dma_start(out=snipt[:], in_=snip[:])

            diagb = constp.tile([128, EC * W, 128], bf16)

            def diag_chunk(c):
                nc.gpsimd.affine_select(
                    out=diagb[:, c * W : (c + 1) * W, :],
                    in_=w2b[:, c * W : (c + 1) * W, None].to_broadcast(
                        [128, W, 128]
                    ),
                    pattern=[[0, W], [-1, 128]],
                    compare_op=AOT.is_equal,
                    fill=0.0,
                    base=0,
                    channel_multiplier=1,
                )

            def idx_chain(b):
                snipf = workp.tile([1, 1], bf16, tag="snipf")
                nc.vector.tensor_copy(snipf[:], snipt[0:1, b : b + 1])
                ps_sn = psumt.tile([128, 512], f32, tag="tp")
                nc.tensor.matmul(
                    out=ps_sn[:, :1],
                    lhsT=ones1[:, :],
                    rhs=snipf[:, :],
                    start=True,
                    stop=True,
                )
                rows_f = workp.tile([128, 1], f32, tag="rowsf")
                nc.vector.tensor_scalar(
                    rows_f[:], ps_sn[:, :1], 128.0, iota_f[:, :1],
                    AOT.mult, AOT.add,
                )
                rows2 = workp.tile([128, 1], i32, tag="rows2")
                nc.vector.tensor_copy(rows2[:], rows_f[:])
                return rows2

            def gather_t2(b, rows2):
                t2raw = workp.tile([128, EC * P], f32, tag="t2raw")
                nc.gpsimd.indirect_dma_start(
                    out=t2raw[:, :],
                    out_offset=None,
                    in_=dataT2[:],
                    in_offset=bass.IndirectOffsetOnAxis(ap=rows2[:, :1], axis=0),
                )
                t2 = workp.tile([128, EC, PPAD], bf16, tag="t2")
                nc.vector.tensor_copy(
                    t2[:, :, 5 : 5 + P],
                    t2raw[:, :].rearrange("p (c q) -> p c q", q=P),
                )
                for c in range(EC):
                    nc.vector.tensor_copy(
                        t2[:, c, 0:5], t2[:, c, 5:6].to_broadcast([128, 5])
                    )
                    nc.vector.tensor_copy(
                        t2[:, c, 5 + P : PPAD],
                        t2[:, c, 4 + P : 5 + P].to_broadcast([128, 5]),
                    )
                return t2

            def onehot(b):
                inpr_f = workp.tile([1, S], bf16, tag="inprf")
                nc.vector.tensor_copy(
                    inpr_f[:], inpt[0:1, b * S : (b + 1) * S]
                )
                inpb_f = workp.tile([128, S], bf16, tag="inpbf")
                for n0 in range(0, S, 512):
                    nw = min(512, S - n0)
                    ps_in = psumt.tile([128, 512], f32, tag="tp")
                    nc.tensor.matmul(
                        out=ps_in[:, :nw],
                        lhsT=ones1[:, :],
                        rhs=inpr_f[:, n0 : n0 + nw],
                        start=True,
                        stop=True,
                    )
                    nc.vector.tensor_copy(
                        inpb_f[:, n0 : n0 + nw], ps_in[:, :nw]
                    )
                oh0 = mmp.tile([128, S], bf16, tag="oh0")
                oh1 = mmp.tile([128, S], bf16, tag="oh1")
                nc.vector.tensor_scalar(
                    oh0[:], inpb_f[:], iota_f[:, :1], None, AOT.is_equal
                )
                nc.vector.tensor_scalar(
                    oh1[:], inpb_f[:], iota_f_hi[:, :1], None, AOT.is_equal
                )
                return oh0, oh1

            # ---- setup / load interleave: affine chunks between gathers
            rows_b = [None] * BPC
            t2_b = [None] * BPC
            rows_b[0] = idx_chain(0)
            diag_chunk(0)
            t2_b[0] = gather_t2(0, rows_b[0])
            rows_b[1] = idx_chain(1)
            diag_chunk(1)
            diag_chunk(2)
            t2_b[1] = gather_t2(1, rows_b[1])
            diag_chunk(3)
            diag_chunk(4)
            diag_chunk(5)
            oh_b = [onehot(0), onehot(1)]

            for b in range(BPC):
                t2 = t2_b[b]
                oh0, oh1 = oh_b[b]

                # ---- 11-tap conv on TensorE, output directly in [p, e]:
                # out[p', e'] = sum_i t2[i, c, off+p'+k] * diag_ck[i, e']
                ccat0 = mmp.tile([128, E], bf16, tag="c0")
                ccat1 = mmp.tile([128, E], bf16, tag="c1")
                nc.vector.memzero(ccat1[:])
                # groups: (pc, c-range, drain engine)
                groups = (
                    (0, range(0, 4), "v"),
                    (0, range(4, EC), "s"),
                    (1, range(0, 4), "s"),
                    (1, range(4, EC), "v"),
                )
                for pc, crange, eng in groups:
                    pcw = 128 if pc == 0 else P - 128
                    gw = len(crange) * 128
                    psc = psumc.tile([128, 512], f32, tag="pc")
                    for ci, c in enumerate(crange):
                        for k in range(W):
                            nc.tensor.matmul(
                                out=psc[:pcw, ci * 128 : (ci + 1) * 128],
                                lhsT=t2[:, c, k + pc * 128 : k + pc * 128 + pcw],
                                rhs=diagb[:, c * W + k, :],
                                start=(k == 0),
                                stop=(k == W - 1),
                            )
                    cdst = ccat0 if pc == 0 else ccat1
                    c0 = crange.start * 128
                    if eng == "v":
                        nc.vector.tensor_copy(
                            cdst[:pcw, c0 : c0 + gw], psc[:pcw, :gw]
                        )
                    else:
                        nc.scalar.copy(
                            cdst[:pcw, c0 : c0 + gw], psc[:pcw, :gw]
                        )
                if debug and b == 0:
                    nc.gpsimd.dma_start(out=dbg["ccat0"][:], in_=ccat0[:])
                    nc.gpsimd.dma_start(out=dbg["oh0"][:], in_=oh0[:])

                # ---- gather matmul: out[s, e] = sum_p oh[p, s] * C[p, e]
                for m in range(MTILES):
                    mw = min(128, S - m * 128)
                    pso = psummm.tile([128, E], f32, tag="po")
                    for oh, cc, st in ((oh0, ccat0, True), (oh1, ccat1, False)):
                        for n0, nw in ((0, 512), (512, 256)):
                            nc.tensor.matmul(
                                out=pso[:mw, n0 : n0 + nw],
                                lhsT=oh[:, m * 128 : m * 128 + mw],
                                rhs=cc[:, n0 : n0 + nw],
                                start=st,
                                stop=not st,
                            )
                    ob = obp.tile([128, E], f32, tag="ob")
                    if m % 2 == 0:
                        nc.vector.tensor_copy(ob[:mw, :], pso[:mw, :])
                    else:
                        nc.scalar.copy(ob[:mw, :], pso[:mw, :])
                    nc.sync.dma_start(
                        out=out[b * S + m * 128 : b * S + m * 128 + mw, :],
                        in_=ob[:mw, :],
                    )

    nc.finalize()
    return nc


def _get_nc():
    if "nc" not in _cache:
        _cache["nc"] = _build()
    return _cache["nc"]


def _prep_shared(data, w):
    # layout-only host staging (no arithmetic)
    d0 = np.asarray(data, dtype=np.float32)[:, :, 0, :]  # [100, 166, 768]
    dT = np.transpose(d0, (0, 2, 1))  # [100, 768, 166]
    dT = (
        dT.reshape(NSNIP, EC, 128, P)
        .transpose(0, 2, 1, 3)
        .reshape(NSNIP * 128, EC * P)
    )
    dataT2 = np.ascontiguousarray(dT, dtype=np.float32)
    wT = np.asarray(w, dtype=np.float32).T  # [768, 11]
    w2 = np.ascontiguousarray(
        wT.reshape(EC, 128, W).transpose(1, 0, 2).reshape(128, EC * W),
        dtype=np.float32,
    )
    return dataT2, w2


def kernel(inputs, code_snippet_id, data, w, _trace=False):
    from concourse.bass_utils import run_bass_kernel_spmd

    nc = _get_nc()
    inputs = np.asarray(inputs, dtype=np.int32)
    code_snippet_id = np.asarray(code_snippet_id, dtype=np.int32)
    dataT2, w2 = _prep_shared(data, w)

    in_maps = []
    for ci in range(N_CORES):
        b0 = ci * BPC
        in_maps.append(
            {
                "inp": np.ascontiguousarray(
                    inputs[b0 : b0 + BPC].reshape(1, BPC * S)
                ),
                "snip": np.ascontiguousarray(
                    code_snippet_id[b0 : b0 + BPC].reshape(1, BPC)
                ),
                "dataT2": dataT2,
                "w2": w2,
            }
        )

    res = run_bass_kernel_spmd(
        nc, in_maps, core_ids=list(range(N_CORES)), trace=_trace
    )
    _cache["last_results"] = res
    out = np.concatenate(
        [res.results[i]["out"].reshape(BPC, S, E) for i in range(N_CORES)],
        axis=0,
    ).astype(np.float32)
    return out
